# revision 4
# baseline (speedup 1.0000x reference)
"""Causal multi-head attention (B=1, S=2048, H=16, D=128, fp32) on 8 TRN2
NeuronCores — head parallelism (2 heads/core), no collectives.

Per-core engine budget (from trace analysis of the 65.4us baseline):
  ACT exp stream ~31us + per-instr overhead  -> the binding engine
  PE  mm1+mm2+l  ~35us
  DVE denominator/normalize ops ~0.4-0.7us each -> keep op COUNT low
Layout: Q/K pre-transposed on host to [d, s] fp16 (contraction on
partitions), V natural [s, h, d] fp16, output [h, d, s] fp16 (host
transposes + upcasts).  fp16 streams the PE at 1 col/cycle like bf16.

Differences vs the 65.4us baseline:
  - flat software pipeline ACROSS s-blocks (lookahead 1 group carries over
    block boundaries; block-end reduction/normalize/DMA no longer bubble
    the ACT stream)
  - block order (1,2,3,0): tail ends on the smallest block
  - input chunks combine both heads into one DMA ([d, h, 512]); issue
    queues: K on sync, Q on vector, V on gpsimd -> first chunks land ~3us
    sooner and issue cost stops serializing on one queue
  - denominator pair-adds moved to gpsimd (DVE was a hidden co-bottleneck);
    accumulate + first-write stay on DVE, all fp16
  - normalize = single DVE divide (psum_o / psum_l) with fp16 output; out
    DMA is fp16 (half the bytes), one DMA per (head, block)
"""

import math

import numpy as np

import concourse.mybir as mybir
import concourse.tile as tile
from concourse import bacc
from concourse.masks import make_upper_triangular

S = 2048
H = 16
D = 128
HC = 2  # heads per core
NCORES = 8
P = 128
SBLK = 512  # s-block width
NT = S // P  # 16 t tiles
NB = S // SBLK  # 4 s blocks / chunks
TPB = SBLK // P  # 4 t tiles per s block
SCALE = 1.0 / math.sqrt(D)

F32 = mybir.dt.float32
FP16 = mybir.dt.float16  # fp16: same PE rate as bf16, 10-bit mantissa

BLOCK_ORDER = (1, 2, 3, 0)
N_WARMUP = 16
# a DVE op can read only ONE operand from PSUM (NCC_IBVF027), so the
# normalize is recip(psum_l)->SBUF then mul(psum_o, recip)->fp16
USE_DIVIDE = False


def _groups_of(b):
    """Groups of two t-tiles sharing one 2-bank psum + one exp:
    (i0, i1, s_lo0, s_lo1, is_diag)."""
    n_full = TPB * b
    gs = [(ip, ip + 1, 0, 0, False) for ip in range(0, n_full, 2)]
    gs += [
        (n_full, n_full + 1, 0, P, True),
        (n_full + 2, n_full + 3, 2 * P, 3 * P, True),
    ]
    return gs


def build_nc():
    nc = bacc.Bacc("TRN2", target_bir_lowering=False, debug=False, num_devices=NCORES)
    qt_d = nc.dram_tensor("qt", [HC, D, S], FP16, kind="ExternalInput").ap()
    kt_d = nc.dram_tensor("kt", [HC, D, S], FP16, kind="ExternalInput").ap()
    v_d = nc.dram_tensor("v", [S, HC, D], FP16, kind="ExternalInput").ap()
    ot_d = nc.dram_tensor("ot", [HC, D, S], FP16, kind="ExternalOutput").ap()

    with tile.TileContext(nc) as tc:
        with (
            tc.tile_pool(name="consts", bufs=1) as cpool,
            tc.tile_pool(name="big", bufs=1) as bigpool,
            tc.tile_pool(name="exp", bufs=8) as epool,
            tc.tile_pool(name="norm", bufs=4) as npool,
            tc.tile_pool(name="psum_s", bufs=2, space="PSUM") as ps_pool,
            tc.tile_pool(name="psum_o", bufs=3, space="PSUM") as po_pool,
            tc.tile_pool(name="psum_l", bufs=1, space="PSUM") as pl_pool,
        ):
            ones = cpool.tile([P, P], FP16, tag="ones")
            nc.vector.memset(ones, 1.0)
            warm_ps = pl_pool.tile([P, SBLK], F32, tag="pl", name="warm_ps")
            for w in range(N_WARMUP):
                nc.tensor.matmul(
                    warm_ps[:, :P],
                    ones[:],
                    ones[:],
                    start=True,
                    stop=True,
                    skip_group_check=True,
                )
            tri = cpool.tile([P, P], FP16, tag="tri")
            make_upper_triangular(nc, tri, val=1.0, diag=True)

            # chunked SBUF inputs: per-chunk K^T/Q^T [d, h, 512] fp16 (both
            # heads in one DMA) and V natural [t-part, j, h, d] fp16 chunks,
            # issued on three queues in the order compute consumes them.
            ktre = kt_d.rearrange("h d s -> d h s")
            qtre = qt_d.rearrange("h d s -> d h s")
            vre = v_d.rearrange("(i p) h d -> p i h d", p=P)
            kt_c = {}
            qt_c = {}
            vb_c = {}
            for c in range(NB):
                kt_c[c] = bigpool.tile([P, HC, SBLK], FP16, tag=f"ktc{c}", name=f"ktc{c}")
                qt_c[c] = bigpool.tile([P, HC, SBLK], FP16, tag=f"qtc{c}", name=f"qtc{c}")
                vb_c[c] = bigpool.tile([P, TPB, HC, D], FP16, tag=f"vbc{c}", name=f"vbc{c}")

            def dma_chunk(eng, dst, src, halves):
                if halves:
                    hw_ = SBLK // 2
                    eng.dma_start(dst[:, :, :hw_], src[:, :, :hw_])
                    eng.dma_start(dst[:, :, hw_:], src[:, :, hw_:])
                else:
                    eng.dma_start(dst[:], src)

            def cs(c):
                return slice(c * SBLK, (c + 1) * SBLK)

            # DMA issue queues (only sync/gpsimd/scalar can issue): K and
            # late Q chunks on sync, first Q chunk + V on gpsimd, in the
            # order the (1,2,3,0) block schedule consumes them.
            dma_chunk(nc.sync, kt_c[0], ktre[:, :, cs(0)], True)
            dma_chunk(nc.gpsimd, qt_c[1], qtre[:, :, cs(1)], True)
            nc.gpsimd.dma_start(vb_c[0][:], vre[:, 0:TPB])
            dma_chunk(nc.sync, kt_c[1], ktre[:, :, cs(1)], False)
            nc.gpsimd.dma_start(vb_c[1][:], vre[:, TPB : 2 * TPB])
            dma_chunk(nc.sync, qt_c[2], qtre[:, :, cs(2)], False)
            dma_chunk(nc.sync, kt_c[2], ktre[:, :, cs(2)], False)
            nc.gpsimd.dma_start(vb_c[2][:], vre[:, 2 * TPB : 3 * TPB])
            dma_chunk(nc.sync, qt_c[3], qtre[:, :, cs(3)], False)
            dma_chunk(nc.sync, kt_c[3], ktre[:, :, cs(3)], False)
            nc.gpsimd.dma_start(vb_c[3][:], vre[:, 3 * TPB : 4 * TPB])
            dma_chunk(nc.sync, qt_c[0], qtre[:, :, cs(0)], False)

            def kt_tile(h, i):
                return kt_c[i // TPB][:, h, (i % TPB) * P : (i % TPB + 1) * P]

            def v_tile(h, i):
                return vb_c[i // TPB][:, i % TPB, h, :]

            # per-block state
            psum_o = {}
            psum_l = {}
            expsum = {}
            expt_of = {}

            def ensure_block(b):
                if (0, b) in psum_o:
                    return
                for h in range(HC):
                    psum_o[h, b] = po_pool.tile(
                        [P, SBLK], F32, tag="po", name=f"po{h}_{b}"
                    )
                    psum_l[h, b] = pl_pool.tile(
                        [P, SBLK], F32, tag="pl", name=f"pl{h}_{b}"
                    )
                    if b:
                        expsum[h, b] = bigpool.tile(
                            [P, SBLK], FP16, tag=f"esum{h}_{b}", name=f"es{h}_{b}"
                        )

            def emit_mm1(h, b, grp):
                i0, i1, s0, s1, is_diag = grp
                psum_s = ps_pool.tile([P, 2, SBLK], F32, tag="ps", name=f"ps{h}_{b}_{i0}")
                expt = epool.tile([P, 2, SBLK], FP16, tag="expt", name=f"ex{h}_{b}_{i0}")
                for j, (i, s_lo) in enumerate(((i0, s0), (i1, s1))):
                    nc.tensor.matmul(
                        psum_s[:, j, s_lo:],
                        kt_tile(h, i),
                        qt_c[b][:, h, s_lo:],
                        start=True,
                        stop=True,
                    )
                # one exp for both tiles; [s0:s1] of tile 1 is stale-finite
                # psum, never read downstream
                nc.scalar.activation(
                    expt[:, :, s0:],
                    psum_s[:, :, s0:],
                    mybir.ActivationFunctionType.Exp,
                    scale=SCALE,
                )
                if is_diag:
                    for j, s_lo in enumerate((s0, s1)):
                        nc.gpsimd.tensor_mul(
                            out=expt[:, j, s_lo : s_lo + P],
                            in0=expt[:, j, s_lo : s_lo + P],
                            in1=tri[:],
                        )
                expt_of[h, b, i0] = expt

            def emit_mm2(h, b, grp):
                i0, i1, s0, s1, is_diag = grp
                n_full = TPB * b
                last_i = n_full + TPB - 1
                expt = expt_of.pop((h, b, i0))
                for j, (i, s_lo) in enumerate(((i0, s0), (i1, s1))):
                    nc.tensor.matmul(
                        psum_o[h, b][:, s_lo:],
                        v_tile(h, i),
                        expt[:, j, s_lo:],
                        start=(i == 0),
                        stop=(i == last_i),
                        skip_group_check=True,
                    )
                    if is_diag:
                        # diagonal denominator contributions on PE
                        nc.tensor.matmul(
                            psum_l[h, b][:, s_lo:],
                            ones[:],
                            expt[:, j, s_lo:],
                            start=(i == n_full),
                            stop=(i == last_i and n_full == 0),
                            skip_group_check=True,
                        )
                if not is_diag:
                    # full-tile denominator: pair-add on gpsimd (except the
                    # first, which writes expsum directly on DVE), fp32-free
                    # fp16 chain accumulated on DVE
                    if i0 == 0:
                        nc.vector.tensor_add(
                            out=expsum[h, b][:],
                            in0=expt[:, 0, :],
                            in1=expt[:, 1, :],
                        )
                    else:
                        pair = npool.tile(
                            [P, SBLK], FP16, tag="epair", name=f"ep{h}_{b}_{i0}"
                        )
                        nc.gpsimd.tensor_add(
                            out=pair[:],
                            in0=expt[:, 0, :],
                            in1=expt[:, 1, :],
                        )
                        nc.vector.tensor_add(
                            out=expsum[h, b][:],
                            in0=expsum[h, b][:],
                            in1=pair[:],
                        )

            def block_end(b):
                for h in range(HC):
                    bs = slice(b * SBLK, (b + 1) * SBLK)
                    if b:
                        # contract the DVE partial sums over the partition dim
                        nc.tensor.matmul(
                            psum_l[h, b][:],
                            ones[:],
                            expsum[h, b][:],
                            start=False,
                            stop=True,
                            skip_group_check=True,
                        )
                    otn = npool.tile([P, SBLK], FP16, tag="otn", name=f"ot{h}_{b}")
                    if USE_DIVIDE:
                        nc.vector.tensor_tensor(
                            out=otn[:],
                            in0=psum_o[h, b][:],
                            in1=psum_l[h, b][:],
                            op=mybir.AluOpType.divide,
                        )
                    else:
                        recip = npool.tile(
                            [P, SBLK], F32, tag="recip", name=f"rc{h}_{b}"
                        )
                        nc.vector.reciprocal_approx_fast(
                            out=recip[:], in_=psum_l[h, b][:]
                        )
                        nc.vector.tensor_mul(
                            out=otn[:], in0=psum_o[h, b][:], in1=recip[:]
                        )
                    nc.sync.dma_start(ot_d[h, :, bs], otn[:])

            # flat pipeline across blocks, lookahead one group
            sched = []
            for b in BLOCK_ORDER:
                gs = _groups_of(b)
                for gi, grp in enumerate(gs):
                    sched.append((b, grp, gi == len(gs) - 1))
            pending = None
            for b, grp, is_last in sched:
                ensure_block(b)
                for h in range(HC):
                    emit_mm1(h, b, grp)
                if pending is not None:
                    pb, pgrp, plast = pending
                    for h in range(HC):
                        emit_mm2(h, pb, pgrp)
                    if plast:
                        block_end(pb)
                pending = (b, grp, is_last)
            pb, pgrp, plast = pending
            for h in range(HC):
                emit_mm2(h, pb, pgrp)
            block_end(pb)
    nc.compile()
    return nc


_NC_CACHE = None


def _get_nc():
    global _NC_CACHE
    if _NC_CACHE is None:
        _NC_CACHE = build_nc()
    return _NC_CACHE


def make_in_maps(query, key, value):
    query = np.asarray(query)
    key = np.asarray(key)
    value = np.asarray(value)
    in_maps = []
    for c in range(NCORES):
        hs = slice(c * HC, (c + 1) * HC)
        in_maps.append(
            {
                "qt": np.ascontiguousarray(
                    query[0, :, hs, :].transpose(1, 2, 0)
                ).astype(np.float16),
                "kt": np.ascontiguousarray(
                    key[0, :, hs, :].transpose(1, 2, 0)
                ).astype(np.float16),
                "v": np.ascontiguousarray(value[0, :, hs, :]).astype(np.float16),
            }
        )
    return in_maps


def kernel(query, key, value):
    from concourse.bass_utils import run_bass_kernel_spmd

    nc = _get_nc()
    in_maps = make_in_maps(query, key, value)
    res = run_bass_kernel_spmd(nc, in_maps, core_ids=list(range(NCORES)))
    out = np.empty((1, S, H, D), dtype=np.float32)
    for c in range(NCORES):
        # ot is [HC, D, S] fp16 -> [S, HC, D] fp32
        out[0, :, c * HC : (c + 1) * HC, :] = (
            res.results[c]["ot"].astype(np.float32).transpose(2, 0, 1)
        )
    return out
